# revision 33
# baseline (speedup 1.0000x reference)
"""Trainium2 Bass kernel for nn_CIFModule (histogram_binning).

Data-parallel over batch: 16 batches -> 8 cores x 2 batches.
Takes FULL inputs, returns FULL outputs (acoustic, alpha, qty_loss).
"""

import sys

sys.path.insert(0, "/opt/trn_rl_repo")

from contextlib import ExitStack

import numpy as np

import concourse.bass as bass
import concourse.bacc as bacc
import concourse.mybir as mybir
import concourse.tile as tile
from concourse.bass import IndirectOffsetOnAxis

F32 = mybir.dt.float32
F32R = mybir.dt.float32r
BF16 = mybir.dt.bfloat16
I32 = mybir.dt.int32

# problem dims (per core after batch sharding)
NB = 2          # batches per core
T = 3000        # fire time steps
C = 32          # fire signal dim
H = 32          # swin freq bins
W = 375         # swin time bins
DSW = 192       # swin dim
NF = 128        # n fires
PP = 32         # polyphony
DP = 32         # pitch token dim
DM = 512        # d_model
HID = 128       # cif hidden
NT = T // 128   # 24 time tiles (wait: 3000/128 = 23.4375) -- handled below
LN_EPS = 1e-5

# 3000 = 23*128 + 56 : use 24 tiles, last tile has 56 rows.
TILE_T = 128
N_TTILES = (T + TILE_T - 1) // TILE_T  # 24
LAST_T = T - (N_TTILES - 1) * TILE_T   # 56

# matmul dtype for the big output matmuls (swin/pitch). float32 is exact but
# 4 cycles/row on the PE; float32r runs at 1 cycle/row for N>=256.
USE_F32R_BIG = True
# use float32r for the small alpha-path matmuls too (dense/proj). These feed
# the alpha>1 threshold so precision matters more; keep fp32 unless measured ok.
USE_F32R_ALPHA = False


def _mm_dt(ap, enable):
    # f32r operands are declared with that dtype at tile/tensor creation;
    # this is now a no-op passthrough.
    return ap


def build_nc():
    nc = bacc.Bacc("TRN2", target_bir_lowering=False, debug=False)

    # ---- per-core inputs ----
    fire = nc.dram_tensor("fire", [NB, T, C], F32, kind="ExternalInput")
    swin = nc.dram_tensor("swin", [NB, H, W, DSW], F32, kind="ExternalInput")
    pitch = nc.dram_tensor("pitch", [NB, NF, PP, DP], F32, kind="ExternalInput")
    convw = nc.dram_tensor("convw", [3, C], F32, kind="ExternalInput")
    lnw = nc.dram_tensor("lnw", [C, 1], F32, kind="ExternalInput")
    lnb = nc.dram_tensor("lnb", [C, 1], F32, kind="ExternalInput")
    dw = nc.dram_tensor("dw", [C, HID], F32, kind="ExternalInput")
    db = nc.dram_tensor("db", [HID, 1], F32, kind="ExternalInput")
    pw = nc.dram_tensor("pw", [HID, 1], F32, kind="ExternalInput")
    projb2 = nc.dram_tensor("projb2", [NB, 1], F32, kind="ExternalInput")
    pw33 = nc.dram_tensor("pw33", [DP + 1, DM], F32R, kind="ExternalInput")
    swA = nc.dram_tensor("swA", [128, DM], F32R, kind="ExternalInput")
    swB65 = nc.dram_tensor("swB65", [DSW - 128 + 1, DM], F32R, kind="ExternalInput")
    ident = nc.dram_tensor("ident", [128, 128], F32, kind="ExternalInput")
    slotv = nc.dram_tensor("slotv", [128, NF], F32, kind="ExternalInput")
    mask8 = nc.dram_tensor("mask8", [128, 1], BF16, kind="ExternalInput")
    hoff = nc.dram_tensor("hoff", [NB, NF, H], I32, kind="ExternalInput")
    zrow = nc.dram_tensor("zrow", [1, C], F32, kind="ExternalInput")
    onesr = nc.dram_tensor("onesr", [1, 128], F32R, kind="ExternalInput")

    # ---- per-core outputs ----
    acoustic = nc.dram_tensor("acoustic", [NB, NF, PP + H, DM], F32,
                              kind="ExternalOutput")
    alpha_o = nc.dram_tensor("alpha_o", [NB, T], F32, kind="ExternalOutput")
    osum = nc.dram_tensor("osum", [NB, 1], F32, kind="ExternalOutput")

    with tile.TileContext(nc) as tc, ExitStack() as ctx:
        const = ctx.enter_context(tc.tile_pool(name="const", bufs=1))
        abuf = ctx.enter_context(tc.tile_pool(name="abuf", bufs=1))
        work = ctx.enter_context(tc.tile_pool(name="work", bufs=3))
        sgp = ctx.enter_context(tc.tile_pool(name="sgp", bufs=2))
        outp = ctx.enter_context(tc.tile_pool(name="outp", bufs=4))
        ps_big = ctx.enter_context(tc.tile_pool(name="ps_big", bufs=3, space="PSUM"))
        ps_tr = ctx.enter_context(tc.tile_pool(name="ps_tr", bufs=2, space="PSUM"))
        ps_sm = ctx.enter_context(tc.tile_pool(name="ps_sm", bufs=2, space="PSUM"))

        # ================= constants =================
        ident_sb = const.tile([128, 128], F32)
        nc.sync.dma_start(out=ident_sb[:], in_=ident[:])
        slotv_sb = const.tile([128, NF], F32)
        nc.sync.dma_start(out=slotv_sb[:], in_=slotv[:])
        mask8_sb = const.tile([128, 1], BF16)
        nc.sync.dma_start(out=mask8_sb[:], in_=mask8[:])
        hoff_sb = const.tile([128, NB * H], I32)
        for b in range(NB):
            nc.sync.dma_start(out=hoff_sb[:, b * H:(b + 1) * H], in_=hoff[b])
        swA_sb = const.tile([128, DM], F32R)
        nc.sync.dma_start(out=swA_sb[:], in_=swA[:])
        swB65_sb = const.tile([65, DM], F32R)
        nc.sync.dma_start(out=swB65_sb[:], in_=swB65[:])
        pw33_sb = const.tile([DP + 1, DM], F32R)
        nc.sync.dma_start(out=pw33_sb[:], in_=pw33[:])
        dw_sb = const.tile([C, HID], F32)
        nc.sync.dma_start(out=dw_sb[:], in_=dw[:])
        lnw_sb = const.tile([C, 1], F32)
        nc.sync.dma_start(out=lnw_sb[:], in_=lnw[:])
        lnb_sb = const.tile([C, 1], F32)
        nc.sync.dma_start(out=lnb_sb[:], in_=lnb[:])
        db_sb = const.tile([HID, 1], F32)
        nc.sync.dma_start(out=db_sb[:], in_=db[:])
        pw_sb = const.tile([HID, 1], F32)
        nc.sync.dma_start(out=pw_sb[:], in_=pw[:])
        projb2_sb = const.tile([NB, 1], F32)
        nc.sync.dma_start(out=projb2_sb[:], in_=projb2[:])

        onesr_sb = const.tile([1, 128], F32R)
        nc.sync.dma_start(out=onesr_sb[:], in_=onesr[:])
        zero128 = const.tile([128, 1], F32)
        nc.vector.memset(zero128[:], 0.0)
        negten = const.tile([NB, 1], F32)
        nc.vector.memset(negten[:], -10.0)

        # conv taps replicated across partitions; [128, 3, 2*C]
        convr = const.tile([128, 3, NB * C], F32)
        for k in range(3):
            for g in range(NB):
                src = bass.AP(tensor=convw, offset=k * C, ap=[[0, 128], [1, C]])
                nc.sync.dma_start(out=convr[:, k, g * C:(g + 1) * C], in_=src)
        # center tap + 1 (residual)
        nc.vector.tensor_scalar_add(convr[:, 1, :], convr[:, 1, :], 1.0)

        # dense weights with LN affine folded in:
        #   dwp = diag(ln_w) @ dw  ;  biasd = dw.T @ ln_b + db
        dwp_sb = const.tile([C, HID], F32)
        nc.vector.tensor_scalar_mul(dwp_sb[:], dw_sb[:], lnw_sb[:])
        psB = ps_sm.tile([HID, 1], F32, tag="sm")
        nc.tensor.matmul(psB[:], lhsT=dw_sb[:], rhs=lnb_sb[:], start=True, stop=True)
        biasd_sb = const.tile([HID, 1], F32)
        nc.vector.tensor_add(biasd_sb[:], psB[:], db_sb[:])

        # ================= phase A: alpha =================
        # load fire signal in [t-tile-part, (ttile, b, c)] layout, plus +-1 shifts
        xf = abuf.tile([128, N_TTILES, NB * C], F32)     # x[t]
        xm = abuf.tile([128, N_TTILES, NB * C], F32)     # x[t-1]
        xp = abuf.tile([128, N_TTILES, NB * C], F32)     # x[t+1]
        fire_h = fire  # DRAM handle
        for b in range(NB):
            cs_ = slice(b * C, (b + 1) * C)
            base = b * T * C
            # x[t]: full 23 tiles + last partial tile of 56 rows
            nc.sync.dma_start(
                out=xf[:, 0:N_TTILES - 1, cs_],
                in_=bass.AP(tensor=fire_h, offset=base,
                            ap=[[C, 128], [128 * C, N_TTILES - 1], [1, C]]))
            nc.sync.dma_start(
                out=xf[0:LAST_T, N_TTILES - 1, cs_],
                in_=bass.AP(tensor=fire_h, offset=base + (N_TTILES - 1) * 128 * C,
                            ap=[[C, LAST_T], [1, C]]))
            # x[t-1]: row0 of tile0 is zero-pad
            nc.vector.memset(xm[0:1, 0, cs_], 0.0)
            nc.sync.dma_start(
                out=xm[1:128, 0, cs_],
                in_=bass.AP(tensor=fire_h, offset=base, ap=[[C, 127], [1, C]]))
            nc.sync.dma_start(
                out=xm[:, 1:N_TTILES - 1, cs_],
                in_=bass.AP(tensor=fire_h, offset=base + 127 * C,
                            ap=[[C, 128], [128 * C, N_TTILES - 2], [1, C]]))
            nc.sync.dma_start(
                out=xm[0:LAST_T, N_TTILES - 1, cs_],
                in_=bass.AP(tensor=fire_h,
                            offset=base + ((N_TTILES - 1) * 128 - 1) * C,
                            ap=[[C, LAST_T], [1, C]]))
            # x[t+1]: last row of last tile is zero-pad
            nc.sync.dma_start(
                out=xp[:, 0:N_TTILES - 1, cs_],
                in_=bass.AP(tensor=fire_h, offset=base + C,
                            ap=[[C, 128], [128 * C, N_TTILES - 1], [1, C]]))
            nc.sync.dma_start(
                out=xp[0:LAST_T - 1, N_TTILES - 1, cs_],
                in_=bass.AP(tensor=fire_h,
                            offset=base + ((N_TTILES - 1) * 128 + 1) * C,
                            ap=[[C, LAST_T - 1], [1, C]]))
            nc.sync.dma_start(out=xp[LAST_T - 1:LAST_T, N_TTILES - 1, cs_],
                              in_=zrow[0:1, 0:C])

        # conv + residual + LN stats per tile
        mvall = abuf.tile([128, N_TTILES, NB, 2], F32)   # (mean, var)
        # rows >= LAST_T of the final tile are never written by bn_aggr but
        # are read (and discarded) by the vectorized rstd pass.
        nc.vector.memset(mvall[:], 0.0)
        yall = abuf.tile([128, N_TTILES, NB * C], F32)   # conv output
        for k in range(N_TTILES):
            nrow = 128 if k < N_TTILES - 1 else LAST_T
            y = yall[0:nrow, k, :]
            t0 = work.tile([128, NB * C], F32, tag="convtmp")
            nc.vector.tensor_mul(y, xf[0:nrow, k, :], convr[0:nrow, 1, :])
            nc.vector.tensor_mul(t0[0:nrow], xm[0:nrow, k, :], convr[0:nrow, 0, :])
            nc.vector.tensor_add(y, y, t0[0:nrow])
            nc.vector.tensor_mul(t0[0:nrow], xp[0:nrow, k, :], convr[0:nrow, 2, :])
            nc.vector.tensor_add(y, y, t0[0:nrow])
            for b in range(NB):
                st6 = work.tile([128, 6], F32, tag="st6")
                nc.vector.bn_stats(st6[0:nrow], yall[0:nrow, k, b * C:(b + 1) * C])
                nc.vector.bn_aggr(mvall[0:nrow, k, b, :], st6[0:nrow])

        # rstd = 1/sqrt(var+eps) with two Newton rounds
        nvw = N_TTILES * NB
        veps = abuf.tile([128, nvw], F32)
        rr = abuf.tile([128, nvw], F32)
        vview = mvall[:, :, :, 1]  # [128, NT, NB] strided view
        nc.vector.tensor_scalar_add(veps[:].rearrange("p (a b) -> p a b", a=N_TTILES),
                                    vview, LN_EPS)
        sq = work.tile([128, nvw], F32, tag="sq")
        nc.scalar.activation(sq[:], veps[:], mybir.ActivationFunctionType.Sqrt,
                             bias=zero128[:], scale=1.0)
        nc.vector.reciprocal(rr[:], sq[:])
        for _ in range(2):
            t1 = work.tile([128, nvw], F32, tag="nt1")
            nc.vector.tensor_mul(t1[:], rr[:], rr[:])
            nc.vector.tensor_mul(t1[:], t1[:], veps[:])
            nc.vector.tensor_scalar(t1[:], t1[:], -0.5, 1.5,
                                    mybir.AluOpType.mult, mybir.AluOpType.add)
            nc.vector.tensor_mul(rr[:], rr[:], t1[:])
        rrv = rr[:].rearrange("p (a b) -> p a b", a=N_TTILES)

        # normalize + transpose -> per-batch zT [C, T] (base partition 0 for matmul)
        zTb = [abuf.tile([C, N_TTILES * 128], F32, tag=f"zT{b}", name=f"zT{b}")
               for b in range(NB)]
        for k in range(N_TTILES):
            nrow = 128 if k < N_TTILES - 1 else LAST_T
            zt = work.tile([128, NB * C], F32, tag="zt")
            if nrow < 128:
                nc.vector.memset(zt[:], 0.0)
            for b in range(NB):
                nc.vector.tensor_scalar(
                    zt[0:nrow, b * C:(b + 1) * C],
                    yall[0:nrow, k, b * C:(b + 1) * C],
                    mvall[0:nrow, k, b, 0:1], rrv[0:nrow, k, b:b + 1],
                    mybir.AluOpType.subtract, mybir.AluOpType.mult)
            # rows nrow:128 of the last tile carry stale-but-finite data; the
            # resulting zT columns >= T are never consumed.
            for b in range(NB):
                ztp = ps_tr.tile([C, 128], F32, tag="tr")
                nc.tensor.transpose(ztp[:], zt[:, b * C:(b + 1) * C], ident_sb[:])
                nc.vector.tensor_copy(zTb[b][:, k * 128:(k + 1) * 128], ztp[:])

        # dense (hid) + relu + proj + softplus -> alpha
        # fires are thresholded on pre-softplus y: alpha>1 <=> y>ln(e-1)
        THETA = float(np.log(np.exp(1.0) - 1.0))
        thetmb = const.tile([1, 1], F32)  # theta - proj_b
        nc.vector.tensor_scalar(thetmb[:], projb2_sb[0:1, :], -1.0, THETA,
                                mybir.AluOpType.mult, mybir.AluOpType.add)
        projb2n = const.tile([1, 1], F32)  # -proj_b
        nc.vector.tensor_scalar_mul(projb2n[:], projb2_sb[0:1, :], -1.0)
        alpha_b = [abuf.tile([1, N_TTILES * 128], F32, name=f"alpha{b}")
                   for b in range(NB)]
        cs_b = [abuf.tile([1, T], F32, name=f"cs{b}") for b in range(NB)]
        NCHUNK = 6  # 6 x 512 = 3072
        for b in range(NB):
            for j in range(NCHUNK):
                cols = slice(j * 512, (j + 1) * 512)
                ncol = min(T, (j + 1) * 512) - j * 512
                hps = ps_big.tile([HID, 512], F32, tag="big")
                nc.tensor.matmul(
                    hps[:], lhsT=_mm_dt(dwp_sb[:], USE_F32R_ALPHA),
                    rhs=_mm_dt(zTb[b][:, cols], USE_F32R_ALPHA),
                    start=True, stop=True)
                hs = work.tile([HID, 512], F32, tag="hs")
                nc.scalar.activation(hs[:], hps[:],
                                     mybir.ActivationFunctionType.Relu,
                                     bias=biasd_sb[:], scale=1.0)
                aps = ps_sm.tile([1, 512], F32, tag="sm")
                nc.tensor.matmul(aps[:], lhsT=_mm_dt(pw_sb[:], USE_F32R_ALPHA),
                                 rhs=_mm_dt(hs[:], USE_F32R_ALPHA),
                                 start=True, stop=True)
                # softplus(y) = -ln(sigmoid(-y)); stage s = sigmoid(-y) now,
                # take ln + negate after all chunks (one table set at a time)
                nc.scalar.activation(alpha_b[b][0:1, cols], aps[:],
                                     mybir.ActivationFunctionType.Sigmoid,
                                     bias=projb2n[:], scale=-1.0)
                if ncol > 0:
                    nc.vector.tensor_scalar(
                        cs_b[b][0:1, j * 512:j * 512 + ncol], aps[:, 0:ncol],
                        thetmb[:], None, mybir.AluOpType.is_gt)

        # tail per batch: l = ln(s) (in place); qty sums from l directly via
        # sigmoid((-l-1)/0.1); alpha = -l (in place); DMA out; fires cumsum.
        for b in range(NB):
            nc.scalar.activation(alpha_b[b][:], alpha_b[b][:],
                                 mybir.ActivationFunctionType.Ln,
                                 bias=zero128[0:1, :], scale=1.0)
        for b in range(NB):
            osum_sb = work.tile([1, 1], F32, tag="osum_sb")
            # dump the sigmoid values into the (now dead) zT buffer; only the
            # accumulated sum is consumed.
            nc.scalar.activation(zTb[b][0:1, 0:T], alpha_b[b][:, 0:T],
                                 mybir.ActivationFunctionType.Sigmoid,
                                 bias=negten[0:1, :], scale=-10.0,
                                 accum_out=osum_sb[:])
            nc.sync.dma_start(out=osum[b:b + 1, :], in_=osum_sb[:])
            nc.vector.tensor_scalar_mul(alpha_b[b][:], alpha_b[b][:], -1.0)
            nc.sync.dma_start(out=alpha_o[b:b + 1, :], in_=alpha_b[b][:, 0:T])
            nc.vector.tensor_tensor_scan(cs_b[b][:], cs_b[b][:], cs_b[b][:], 0.0,
                                         mybir.AluOpType.add,
                                         mybir.AluOpType.bypass)

        # transpose cumsum -> [t-part, (tile, b)]
        cstp = ps_sm.tile([128, N_TTILES * NB], F32, tag="sm")
        for k in range(N_TTILES):
            ncol = 128 if k < N_TTILES - 1 else LAST_T
            for b in range(NB):
                nc.tensor.transpose(cstp[0:ncol, k * NB + b:k * NB + b + 1],
                                    cs_b[b][:, k * 128:k * 128 + ncol],
                                    ident_sb[0:1, 0:1])
        csT = abuf.tile([128, N_TTILES * NB], F32)
        # rows of the last (short) tile beyond LAST_T must not contribute:
        # pre-fill with large cumsum so slot > cs is false there.
        nc.vector.memset(csT[:], 1e9)
        nc.vector.tensor_copy(csT[:, 0:(N_TTILES - 1) * NB],
                              cstp[:, 0:(N_TTILES - 1) * NB])
        nc.vector.tensor_copy(csT[0:LAST_T, (N_TTILES - 1) * NB:N_TTILES * NB],
                              cstp[0:LAST_T, (N_TTILES - 1) * NB:N_TTILES * NB])

        # fire_w = min(floor(searchsorted/8), 374) via mask8 matmul counts.
        # one accumulation group per batch; bufs=1 serializes the two groups.
        idxs = []
        for b in range(NB):
            fwp = ps_sm.tile([NF, 1], F32, tag="fwacc", bufs=1, name=f"fwp{b}")
            for k in range(N_TTILES):
                isl = work.tile([128, NF], BF16, tag="isl")
                nc.vector.tensor_tensor(
                    isl[:], slotv_sb[:], csT[:, k * NB + b:k * NB + b + 1]
                    .to_broadcast([128, NF]), mybir.AluOpType.is_gt)
                nc.tensor.matmul(fwp[:], lhsT=isl[:], rhs=mask8_sb[:],
                                 start=(k == 0), stop=(k == N_TTILES - 1))
            fwf = work.tile([NF, 1], F32, tag="fwf")
            nc.vector.tensor_scalar_min(fwf[:], fwp[:], 374.0)
            fwi = work.tile([NF, 1], I32, tag="fwi")
            nc.vector.tensor_copy(fwi[:], fwf[:])
            idx_sb = sgp.tile([NF, H], I32, tag="idx")
            nc.vector.tensor_tensor(idx_sb[:], hoff_sb[:, b * H:(b + 1) * H],
                                    fwi[:].to_broadcast([NF, H]),
                                    mybir.AluOpType.add)
            idxs.append(idx_sb)

        # ================= phase C: pitch tokens =================
        for b in range(NB):
            pfull = sgp.tile([128, PP, DP], F32, tag="pfull")
            nc.sync.dma_start(
                out=pfull[:],
                in_=bass.AP(tensor=pitch, offset=b * NF * PP * DP,
                            ap=[[DP, 128], [128 * DP, PP], [1, DP]]))
            for blk in range(32):
                xtp = ps_tr.tile([DP, 128], F32, tag="tr")
                nc.tensor.transpose(xtp[:], pfull[:, blk, :], ident_sb[:])
                xt33 = work.tile([DP + 1, 128], F32R, tag="xt33")
                nc.vector.tensor_copy(xt33[0:DP, :], xtp[:])
                nc.vector.tensor_copy(xt33[DP:DP + 1, :], onesr_sb[:])
                ppps = ps_big.tile([128, DM], F32, tag="big")
                nc.tensor.matmul(ppps[:], lhsT=_mm_dt(xt33[:], USE_F32R_BIG),
                                 rhs=_mm_dt(pw33_sb[:], USE_F32R_BIG),
                                 start=True, stop=True)
                ppo = outp.tile([128, DM], F32, tag="out")
                nc.vector.tensor_copy(ppo[:], ppps[:])
                nc.sync.dma_start(
                    out=bass.AP(tensor=acoustic,
                                offset=(b * NF + blk * 4) * (PP + H) * DM,
                                ap=[[(PP + H) * DM, 4], [DM, PP], [1, DM]]),
                    in_=ppo[:])

        # ================= phase B: swin =================
        swin_flat = swin.ap().rearrange("b h w d -> (b h w) d")
        # HW indirect DMA consumes exactly one index per output partition, so
        # gather one h-slice ([128 fires, 192]) per call.
        for b in range(NB):
            for h in range(H):
                sg = sgp.tile([NF, DSW], F32, tag="sg", name=f"sg{b}_{h}", bufs=6)
                nc.gpsimd.indirect_dma_start(
                    out=sg[:], out_offset=None, in_=swin_flat,
                    in_offset=IndirectOffsetOnAxis(
                        ap=idxs[b][:, h:h + 1], axis=0))
                st1p = ps_tr.tile([128, 128], F32, tag="tr")
                nc.tensor.transpose(st1p[:], sg[:, 0:128], ident_sb[:])
                st2p = ps_tr.tile([64, 128], F32, tag="tr")
                nc.tensor.transpose(st2p[:], sg[:, 128:DSW], ident_sb[:])
                st1s = work.tile([128, 128], F32R, tag="st1s")
                nc.vector.tensor_copy(st1s[:], st1p[:])
                st65 = work.tile([65, 128], F32R, tag="st65")
                nc.vector.tensor_copy(st65[0:64, :], st2p[:])
                nc.vector.tensor_copy(st65[64:65, :], onesr_sb[:])
                swps = ps_big.tile([NF, DM], F32, tag="big")
                nc.tensor.matmul(swps[:], lhsT=_mm_dt(st1s[:], USE_F32R_BIG),
                                 rhs=_mm_dt(swA_sb[:], USE_F32R_BIG),
                                 start=True, stop=False)
                nc.tensor.matmul(swps[:], lhsT=_mm_dt(st65[:], USE_F32R_BIG),
                                 rhs=_mm_dt(swB65_sb[:], USE_F32R_BIG),
                                 start=False, stop=True)
                swo = outp.tile([NF, DM], F32, tag="out")
                nc.vector.tensor_copy(swo[:], swps[:])
                nc.sync.dma_start(
                    out=bass.AP(tensor=acoustic,
                                offset=(b * NF * (PP + H) + PP + h) * DM,
                                ap=[[(PP + H) * DM, NF], [1, DM]]),
                    in_=swo[:])

    nc.finalize()
    return nc


_NC_CACHE = None


def _get_nc():
    global _NC_CACHE
    if _NC_CACHE is None:
        _NC_CACHE = build_nc()
    return _NC_CACHE


def make_constants():
    ident = np.eye(128, dtype=np.float32)
    slotv = np.broadcast_to(np.arange(1, NF + 1, dtype=np.float32)[None, :],
                            (128, NF)).copy()
    import ml_dtypes
    m8 = np.zeros((128, 1), dtype=ml_dtypes.bfloat16)
    m8[7::8, 0] = 1.0
    hoff = np.zeros((NB, NF, H), dtype=np.int32)
    for b in range(NB):
        hoff[b, :, :] = (W * (H * b + np.arange(H)))[None, :]
    return ident, slotv, m8, hoff


def kernel(fire_signal, swin_2d, pitch_tokens, target_lengths,
           conv_w, ln_w, ln_b, dense_w, dense_b, proj_w, proj_b,
           pitch_w, pitch_b, swin_w, swin_b):
    from concourse.bass_utils import run_bass_kernel_spmd

    target_lengths = np.asarray(target_lengths)
    inputs = dict(fire_signal=fire_signal, swin_2d=swin_2d,
                  pitch_tokens=pitch_tokens, conv_w=conv_w, ln_w=ln_w,
                  ln_b=ln_b, dense_w=dense_w, dense_b=dense_b, proj_w=proj_w,
                  proj_b=proj_b, pitch_w=pitch_w, pitch_b=pitch_b,
                  swin_w=swin_w, swin_b=swin_b)
    in_maps = _build_in_maps(inputs)
    n_cores = 8

    nc = _get_nc()
    res = run_bass_kernel_spmd(nc, in_maps, list(range(n_cores)))
    outs = res.results

    acoustic = np.concatenate([outs[c]["acoustic"] for c in range(n_cores)], axis=0)
    alpha = np.concatenate([outs[c]["alpha_o"] for c in range(n_cores)], axis=0)
    osums = np.concatenate([outs[c]["osum"][:, 0] for c in range(n_cores)], axis=0)
    qty = np.float32(np.mean(np.abs(osums - target_lengths.astype(np.float32))))
    return acoustic, alpha, qty


def _build_in_maps(inputs):
    """Shard + pack full inputs into per-core in_maps (same as kernel())."""
    fire_signal = np.asarray(inputs["fire_signal"], np.float32)
    conv_w = np.asarray(inputs["conv_w"], np.float32)
    ident, slotv, m8, hoff = make_constants()
    convw_t = np.ascontiguousarray(conv_w[:, 0, :].T)
    base = {
        "convw": convw_t,
        "lnw": np.asarray(inputs["ln_w"], np.float32).reshape(C, 1),
        "lnb": np.asarray(inputs["ln_b"], np.float32).reshape(C, 1),
        "dw": np.ascontiguousarray(np.asarray(inputs["dense_w"], np.float32)),
        "db": np.asarray(inputs["dense_b"], np.float32).reshape(HID, 1),
        "pw": np.ascontiguousarray(np.asarray(inputs["proj_w"], np.float32)).reshape(HID, 1),
        "projb2": np.full((NB, 1), np.float32(np.asarray(inputs["proj_b"]).reshape(-1)[0]), np.float32),
        "pw33": np.concatenate([np.asarray(inputs["pitch_w"], np.float32),
                                np.asarray(inputs["pitch_b"], np.float32).reshape(1, DM)], 0),
        "swA": np.ascontiguousarray(np.asarray(inputs["swin_w"], np.float32)[0:128]),
        "swB65": np.concatenate([np.asarray(inputs["swin_w"], np.float32)[128:DSW],
                                 np.asarray(inputs["swin_b"], np.float32).reshape(1, DM)], 0),
        "ident": ident, "slotv": slotv, "mask8": m8, "hoff": hoff,
        "zrow": np.zeros((1, C), np.float32),
        "onesr": np.ones((1, 128), np.float32),
    }
    maps = []
    for c in range(8):
        bs = slice(c * NB, (c + 1) * NB)
        m = dict(base)
        m["fire"] = np.ascontiguousarray(fire_signal[bs])
        m["swin"] = np.ascontiguousarray(np.asarray(inputs["swin_2d"], np.float32)[bs])
        m["pitch"] = np.ascontiguousarray(np.asarray(inputs["pitch_tokens"], np.float32)[bs])
        maps.append(m)
    return maps


def timed_run(inputs, iters=6):
    """Steady-state per-launch wall time of the 8-core SPMD kernel, in ns.

    Jits once, keeps inputs on device, feeds each run's outputs back as the
    next run's donated output buffers so no host transfers land in the timed
    region.
    """
    import time
    import jax
    from jax.sharding import Mesh, PartitionSpec
    from jax.experimental.shard_map import shard_map
    from concourse import bass2jax
    from concourse.bass2jax import _bass_exec_p, partition_id_tensor
    import concourse.mybir as mybir_

    nc = _get_nc()
    bass2jax.install_neuronx_cc_hook()
    in_maps = _build_in_maps(inputs)
    n_cores = 8

    partition_name = nc.partition_id_tensor.name if nc.partition_id_tensor else None
    in_names, out_names, out_avals, zero_outs = [], [], [], []
    for alloc in nc.m.functions[0].allocations:
        if not isinstance(alloc, mybir_.MemoryLocationSet):
            continue
        name = alloc.memorylocations[0].name
        if alloc.kind == "ExternalInput":
            if name != partition_name:
                in_names.append(name)
        elif alloc.kind == "ExternalOutput":
            shape = tuple(alloc.tensor_shape)
            dtype = mybir_.dt.np(alloc.dtype)
            out_names.append(name)
            out_avals.append(jax.core.ShapedArray(shape, dtype))
            zero_outs.append(np.zeros(shape, dtype))
    n_params = len(in_names)
    n_outs = len(out_avals)
    all_in_names = in_names + out_names + ([partition_name] if partition_name else [])

    def _body(*args):
        operands = list(args)
        if partition_name is not None:
            operands.append(partition_id_tensor())
        outs = _bass_exec_p.bind(
            *operands, out_avals=tuple(out_avals), in_names=tuple(all_in_names),
            out_names=tuple(out_names), lowering_input_output_aliases=(),
            sim_require_finite=True, sim_require_nnan=True, nc=nc)
        return tuple(outs)

    devices = jax.devices()[:n_cores]
    mesh = Mesh(np.asarray(devices), ("core",))
    in_specs = (PartitionSpec("core"),) * (n_params + n_outs)
    out_specs = (PartitionSpec("core"),) * n_outs
    donate = tuple(range(n_params, n_params + n_outs))
    sharded = jax.jit(
        shard_map(_body, mesh=mesh, in_specs=in_specs, out_specs=out_specs,
                  check_rep=False),
        donate_argnums=donate, keep_unused=True)

    concat_in = [np.concatenate([np.asarray(in_maps[c][n]) for c in range(n_cores)], 0)
                 for n in in_names]
    cur_outs = [np.zeros((n_cores * z.shape[0], *z.shape[1:]), z.dtype)
                for z in zero_outs]
    sharding = jax.sharding.NamedSharding(mesh, PartitionSpec("core"))
    dev_in = [jax.device_put(a, sharding) for a in concat_in]
    cur_outs = [jax.device_put(a, sharding) for a in cur_outs]

    times = []
    for i in range(iters):
        t0 = time.perf_counter()
        res = sharded(*dev_in, *cur_outs)
        jax.block_until_ready(res)
        t1 = time.perf_counter()
        times.append(t1 - t0)
        cur_outs = list(res)
    times = sorted(times[1:])  # drop compile/warmup iteration
    med = times[len(times) // 2]
    return int(med * 1e9)


# revision 46
# speedup vs baseline: 1.3377x; 1.3377x over previous
"""Trainium2 Bass kernel for nn_CIFModule (histogram_binning).

Data-parallel over batch: 16 batches -> 8 cores x 2 batches.
Takes FULL inputs, returns FULL outputs (acoustic, alpha, qty_loss).
"""

import sys

sys.path.insert(0, "/opt/trn_rl_repo")

from contextlib import ExitStack

import numpy as np

import concourse.bass as bass
import concourse.bacc as bacc
import concourse.mybir as mybir
import concourse.tile as tile
from concourse.bass import IndirectOffsetOnAxis

F32 = mybir.dt.float32
F32R = mybir.dt.float32r
BF16 = mybir.dt.bfloat16
I32 = mybir.dt.int32

# problem dims (per core after batch sharding)
NB = 2          # batches per core
T = 3000        # fire time steps
C = 32          # fire signal dim
H = 32          # swin freq bins
W = 375         # swin time bins
DSW = 192       # swin dim
NF = 128        # n fires
PP = 32         # polyphony
DP = 32         # pitch token dim
DM = 512        # d_model
HID = 128       # cif hidden
NT = T // 128   # 24 time tiles (wait: 3000/128 = 23.4375) -- handled below
LN_EPS = 1e-5

# 3000 = 23*128 + 56 : use 24 tiles, last tile has 56 rows.
TILE_T = 128
N_TTILES = (T + TILE_T - 1) // TILE_T  # 24
LAST_T = T - (N_TTILES - 1) * TILE_T   # 56

# matmul dtype for the big output matmuls (swin/pitch). float32 is exact but
# 4 cycles/row on the PE; float32r runs at 1 cycle/row for N>=256.
USE_F32R_BIG = True
# use float32r for the small alpha-path matmuls too (dense/proj). These feed
# the alpha>1 threshold so precision matters more; keep fp32 unless measured ok.
USE_F32R_ALPHA = False


def _mm_dt(ap, enable):
    # f32r operands are declared with that dtype at tile/tensor creation;
    # this is now a no-op passthrough.
    return ap


def build_nc(skip_swin=False, skip_pitch=False, skip_alpha=False):
    nc = bacc.Bacc("TRN2", target_bir_lowering=False, debug=False)

    # ---- per-core inputs ----
    fire = nc.dram_tensor("fire", [NB, T, C], F32, kind="ExternalInput")
    swin = nc.dram_tensor("swin", [NB, H, W, DSW], F32, kind="ExternalInput")
    pitch = nc.dram_tensor("pitch", [NB, NF, PP, DP], F32, kind="ExternalInput")
    convw = nc.dram_tensor("convw", [3, C], F32, kind="ExternalInput")
    lnw = nc.dram_tensor("lnw", [C, 1], F32, kind="ExternalInput")
    lnb = nc.dram_tensor("lnb", [C, 1], F32, kind="ExternalInput")
    dw = nc.dram_tensor("dw", [C, HID], F32, kind="ExternalInput")
    db = nc.dram_tensor("db", [HID, 1], F32, kind="ExternalInput")
    pw = nc.dram_tensor("pw", [HID, 1], F32, kind="ExternalInput")
    projb2 = nc.dram_tensor("projb2", [NB, 1], F32, kind="ExternalInput")
    pw33 = nc.dram_tensor("pw33", [DP + 1, DM], F32R, kind="ExternalInput")
    swA = nc.dram_tensor("swA", [128, DM], F32R, kind="ExternalInput")
    swB65 = nc.dram_tensor("swB65", [DSW - 128 + 1, DM], F32R, kind="ExternalInput")
    ident = nc.dram_tensor("ident", [128, 128], F32, kind="ExternalInput")
    slotv = nc.dram_tensor("slotv", [128, NF], F32, kind="ExternalInput")
    mask8 = nc.dram_tensor("mask8", [128, 1], BF16, kind="ExternalInput")
    hoff = nc.dram_tensor("hoff", [NB, NF, H], I32, kind="ExternalInput")
    zrow = nc.dram_tensor("zrow", [1, C], F32, kind="ExternalInput")
    onesr = nc.dram_tensor("onesr", [1, 128], F32R, kind="ExternalInput")

    # ---- per-core outputs ----
    acoustic = nc.dram_tensor("acoustic", [NB, NF, PP + H, DM], F32,
                              kind="ExternalOutput")
    alpha_o = nc.dram_tensor("alpha_o", [NB, T], F32, kind="ExternalOutput")
    osum = nc.dram_tensor("osum", [NB, 1], F32, kind="ExternalOutput")

    with tile.TileContext(nc) as tc, ExitStack() as ctx:
        const = ctx.enter_context(tc.tile_pool(name="const", bufs=1))
        abuf = ctx.enter_context(tc.tile_pool(name="abuf", bufs=1))
        work = ctx.enter_context(tc.tile_pool(name="work", bufs=4))
        sgp = ctx.enter_context(tc.tile_pool(name="sgp", bufs=2))
        outp = ctx.enter_context(tc.tile_pool(name="outp", bufs=8))
        ps_big = ctx.enter_context(tc.tile_pool(name="ps_big", bufs=3, space="PSUM"))
        ps_tr = ctx.enter_context(tc.tile_pool(name="ps_tr", bufs=2, space="PSUM"))
        ps_sm = ctx.enter_context(tc.tile_pool(name="ps_sm", bufs=2, space="PSUM"))

        # ================= constants =================
        ident_sb = const.tile([128, 128], F32)
        nc.sync.dma_start(out=ident_sb[:], in_=ident[:])
        slotv_sb = const.tile([128, NF], F32)
        nc.sync.dma_start(out=slotv_sb[:], in_=slotv[:])
        mask8_sb = const.tile([128, 1], BF16)
        nc.sync.dma_start(out=mask8_sb[:], in_=mask8[:])
        hoff_sb = const.tile([128, NB * H], I32)
        for b in range(NB):
            nc.sync.dma_start(out=hoff_sb[:, b * H:(b + 1) * H], in_=hoff[b])
        swA_sb = const.tile([128, DM], F32R)
        nc.sync.dma_start(out=swA_sb[:], in_=swA[:])
        swB65_sb = const.tile([65, DM], F32R)
        nc.sync.dma_start(out=swB65_sb[:], in_=swB65[:])
        pw33_sb = const.tile([DP + 1, DM], F32R)
        nc.sync.dma_start(out=pw33_sb[:], in_=pw33[:])
        dw_sb = const.tile([C, HID], F32)
        nc.sync.dma_start(out=dw_sb[:], in_=dw[:])
        lnw_sb = const.tile([C, 1], F32)
        nc.sync.dma_start(out=lnw_sb[:], in_=lnw[:])
        lnb_sb = const.tile([C, 1], F32)
        nc.sync.dma_start(out=lnb_sb[:], in_=lnb[:])
        db_sb = const.tile([HID, 1], F32)
        nc.sync.dma_start(out=db_sb[:], in_=db[:])
        pw_sb = const.tile([HID, 1], F32)
        nc.sync.dma_start(out=pw_sb[:], in_=pw[:])
        projb2_sb = const.tile([NB, 1], F32)
        nc.sync.dma_start(out=projb2_sb[:], in_=projb2[:])

        onesr_sb = const.tile([1, 128], F32R)
        nc.sync.dma_start(out=onesr_sb[:], in_=onesr[:])
        zero128 = const.tile([128, 1], F32)
        nc.vector.memset(zero128[:], 0.0)
        negten = const.tile([NB, 1], F32)
        nc.vector.memset(negten[:], -10.0)

        # conv taps replicated across partitions; [128, 3, 2*C]
        convr = const.tile([128, 3, NB * C], F32)
        for k in range(3):
            for g in range(NB):
                src = bass.AP(tensor=convw, offset=k * C, ap=[[0, 128], [1, C]])
                nc.sync.dma_start(out=convr[:, k, g * C:(g + 1) * C], in_=src)
        # center tap + 1 (residual)
        nc.vector.tensor_scalar_add(convr[:, 1, :], convr[:, 1, :], 1.0)

        # dense weights with LN affine folded in:
        #   dwp = diag(ln_w) @ dw  ;  biasd = dw.T @ ln_b + db
        dwp_sb = const.tile([C, HID], F32)
        nc.vector.tensor_scalar_mul(dwp_sb[:], dw_sb[:], lnw_sb[:])
        psB = ps_sm.tile([HID, 1], F32, tag="sm")
        nc.tensor.matmul(psB[:], lhsT=dw_sb[:], rhs=lnb_sb[:], start=True, stop=True)
        biasd_sb = const.tile([HID, 1], F32)
        nc.vector.tensor_add(biasd_sb[:], psB[:], db_sb[:])

        # ================= phase C: pitch tokens =================
        for b in (() if skip_pitch else range(NB)):
            pfull = sgp.tile([128, PP, DP], F32, tag="pfull")
            nc.sync.dma_start(
                out=pfull[:],
                in_=bass.AP(tensor=pitch, offset=b * NF * PP * DP,
                            ap=[[DP, 128], [128 * DP, PP], [1, DP]]))
            for blk in range(32):
                xtp = ps_tr.tile([DP, 128], F32, tag="tr")
                nc.tensor.transpose(xtp[:], pfull[:, blk, :], ident_sb[:])
                xt33 = work.tile([DP + 1, 128], F32R, tag="xt33")
                nc.vector.tensor_copy(xt33[0:DP, :], xtp[:])
                nc.vector.tensor_copy(xt33[DP:DP + 1, :], onesr_sb[:])
                ppps = ps_big.tile([128, DM], F32, tag="big")
                nc.tensor.matmul(ppps[:], lhsT=_mm_dt(xt33[:], USE_F32R_BIG),
                                 rhs=_mm_dt(pw33_sb[:], USE_F32R_BIG),
                                 start=True, stop=True)
                ppo = outp.tile([128, DM], F32, tag="out")
                nc.scalar.copy(ppo[:], ppps[:])
                nc.sync.dma_start(
                    out=bass.AP(tensor=acoustic,
                                offset=(b * NF + blk * 4) * (PP + H) * DM,
                                ap=[[(PP + H) * DM, 4], [DM, PP], [1, DM]]),
                    in_=ppo[:])

        # ================= phase A: alpha =================
        # load fire signal in [t-tile-part, (ttile, b, c)] layout, plus +-1 shifts
        xf = abuf.tile([128, N_TTILES, NB * C], F32)     # x[t]
        xm = abuf.tile([128, N_TTILES, NB * C], F32)     # x[t-1]
        xp = abuf.tile([128, N_TTILES, NB * C], F32)     # x[t+1]
        for tt in (xf, xm, xp):
            nc.vector.memset(tt[:, N_TTILES - 1, :], 0.0)
        fire_h = fire  # DRAM handle
        for b in range(NB):
            cs_ = slice(b * C, (b + 1) * C)
            base = b * T * C
            # x[t]: full 23 tiles + last partial tile of 56 rows
            nc.sync.dma_start(
                out=xf[:, 0:N_TTILES - 1, cs_],
                in_=bass.AP(tensor=fire_h, offset=base,
                            ap=[[C, 128], [128 * C, N_TTILES - 1], [1, C]]))
            nc.sync.dma_start(
                out=xf[0:LAST_T, N_TTILES - 1, cs_],
                in_=bass.AP(tensor=fire_h, offset=base + (N_TTILES - 1) * 128 * C,
                            ap=[[C, LAST_T], [1, C]]))
            # x[t-1]: row0 of tile0 is zero-pad
            nc.vector.memset(xm[0:1, 0, cs_], 0.0)
            nc.sync.dma_start(
                out=xm[1:128, 0, cs_],
                in_=bass.AP(tensor=fire_h, offset=base, ap=[[C, 127], [1, C]]))
            nc.sync.dma_start(
                out=xm[:, 1:N_TTILES - 1, cs_],
                in_=bass.AP(tensor=fire_h, offset=base + 127 * C,
                            ap=[[C, 128], [128 * C, N_TTILES - 2], [1, C]]))
            nc.sync.dma_start(
                out=xm[0:LAST_T, N_TTILES - 1, cs_],
                in_=bass.AP(tensor=fire_h,
                            offset=base + ((N_TTILES - 1) * 128 - 1) * C,
                            ap=[[C, LAST_T], [1, C]]))
            # x[t+1]: last row of last tile is zero-pad
            nc.sync.dma_start(
                out=xp[:, 0:N_TTILES - 1, cs_],
                in_=bass.AP(tensor=fire_h, offset=base + C,
                            ap=[[C, 128], [128 * C, N_TTILES - 1], [1, C]]))
            nc.sync.dma_start(
                out=xp[0:LAST_T - 1, N_TTILES - 1, cs_],
                in_=bass.AP(tensor=fire_h,
                            offset=base + ((N_TTILES - 1) * 128 + 1) * C,
                            ap=[[C, LAST_T - 1], [1, C]]))
            nc.sync.dma_start(out=xp[LAST_T - 1:LAST_T, N_TTILES - 1, cs_],
                              in_=zrow[0:1, 0:C])

        # conv + residual over the whole T range in 5 fused ops (the last
        # tile's pad rows hold zeros so the fused reads are initialized),
        # then LN stats per tile.
        mvall = abuf.tile([128, N_TTILES, NB, 2], F32)   # (mean, var)
        # rows >= LAST_T of the final tile are never written by bn_aggr but
        # are read (and discarded) by the vectorized rstd pass.
        nc.vector.memset(mvall[:], 0.0)
        yall = abuf.tile([128, N_TTILES, NB * C], F32)   # conv output
        tap = [convr[:, k3, :].unsqueeze(1).to_broadcast([128, N_TTILES, NB * C])
               for k3 in range(3)]
        ct0 = abuf.tile([128, N_TTILES, NB * C], F32)
        nc.vector.tensor_mul(ct0[:], xm[:], tap[0])
        nc.gpsimd.tensor_mul(yall[:], xf[:], tap[1])
        nc.vector.tensor_add(yall[:], yall[:], ct0[:])
        nc.vector.tensor_mul(ct0[:], xp[:], tap[2])
        nc.vector.tensor_add(yall[:], yall[:], ct0[:])
        for k in range(N_TTILES):
            nrow = 128 if k < N_TTILES - 1 else LAST_T
            for b in range(NB):
                st6 = work.tile([128, 6], F32, tag="st6")
                nc.vector.bn_stats(st6[0:nrow], yall[0:nrow, k, b * C:(b + 1) * C])
                nc.vector.bn_aggr(mvall[0:nrow, k, b, :], st6[0:nrow])

        # rstd = 1/sqrt(var+eps) with two Newton rounds
        nvw = N_TTILES * NB
        veps = abuf.tile([128, nvw], F32)
        rr = abuf.tile([128, nvw], F32)
        vview = mvall[:, :, :, 1]  # [128, NT, NB] strided view
        nc.vector.tensor_scalar_add(veps[:].rearrange("p (a b) -> p a b", a=N_TTILES),
                                    vview, LN_EPS)
        sq = work.tile([128, nvw], F32, tag="sq")
        nc.scalar.activation(sq[:], veps[:], mybir.ActivationFunctionType.Sqrt,
                             bias=zero128[:], scale=1.0)
        nc.vector.reciprocal(rr[:], sq[:])
        for _ in range(2):
            t1 = work.tile([128, nvw], F32, tag="nt1")
            nc.vector.tensor_mul(t1[:], rr[:], rr[:])
            nc.vector.tensor_mul(t1[:], t1[:], veps[:])
            nc.vector.tensor_scalar(t1[:], t1[:], -0.5, 1.5,
                                    mybir.AluOpType.mult, mybir.AluOpType.add)
            nc.vector.tensor_mul(rr[:], rr[:], t1[:])
        rrv = rr[:].rearrange("p (a b) -> p a b", a=N_TTILES)

        # normalize + transpose -> per-batch zT [C, T] (base partition 0 for matmul)
        zTb = [abuf.tile([C, N_TTILES * 128], F32, tag=f"zT{b}", name=f"zT{b}")
               for b in range(NB)]
        for k in range(N_TTILES):
            nrow = 128 if k < N_TTILES - 1 else LAST_T
            zt = work.tile([128, NB * C], F32, tag="zt")
            if nrow < 128:
                nc.vector.memset(zt[:], 0.0)
            for b in range(NB):
                nc.vector.tensor_scalar(
                    zt[0:nrow, b * C:(b + 1) * C],
                    yall[0:nrow, k, b * C:(b + 1) * C],
                    mvall[0:nrow, k, b, 0:1], rrv[0:nrow, k, b:b + 1],
                    mybir.AluOpType.subtract, mybir.AluOpType.mult)
            # rows nrow:128 of the last tile carry stale-but-finite data; the
            # resulting zT columns >= T are never consumed.
            for b in range(NB):
                ztp = ps_tr.tile([C, 128], F32, tag="tr")
                nc.tensor.transpose(ztp[:], zt[:, b * C:(b + 1) * C], ident_sb[:])
                nc.vector.tensor_copy(zTb[b][:, k * 128:(k + 1) * 128], ztp[:])

        # dense (hid) + relu + proj + softplus -> alpha
        # fires are thresholded on pre-softplus y: alpha>1 <=> y>ln(e-1)
        THETA = float(np.log(np.exp(1.0) - 1.0))
        thetmb = const.tile([1, 1], F32)  # theta - proj_b
        nc.vector.tensor_scalar(thetmb[:], projb2_sb[0:1, :], -1.0, THETA,
                                mybir.AluOpType.mult, mybir.AluOpType.add)
        projb2n = const.tile([1, 1], F32)  # -proj_b
        nc.vector.tensor_scalar_mul(projb2n[:], projb2_sb[0:1, :], -1.0)
        alpha_b = [abuf.tile([1, N_TTILES * 128], F32, name=f"alpha{b}")
                   for b in range(NB)]
        cs_b = [abuf.tile([1, T], F32, name=f"cs{b}") for b in range(NB)]
        NCHUNK = 6  # 6 x 512 = 3072
        for b in range(NB):
            for j in range(NCHUNK):
                cols = slice(j * 512, (j + 1) * 512)
                ncol = min(T, (j + 1) * 512) - j * 512
                hps = ps_big.tile([HID, 512], F32, tag="big")
                nc.tensor.matmul(
                    hps[:], lhsT=_mm_dt(dwp_sb[:], USE_F32R_ALPHA),
                    rhs=_mm_dt(zTb[b][:, cols], USE_F32R_ALPHA),
                    start=True, stop=True)
                hs = work.tile([HID, 512], F32, tag="hs")
                nc.scalar.activation(hs[:], hps[:],
                                     mybir.ActivationFunctionType.Relu,
                                     bias=biasd_sb[:], scale=1.0)
                aps = ps_sm.tile([1, 512], F32, tag="sm")
                nc.tensor.matmul(aps[:], lhsT=_mm_dt(pw_sb[:], USE_F32R_ALPHA),
                                 rhs=_mm_dt(hs[:], USE_F32R_ALPHA),
                                 start=True, stop=True)
                # softplus(y) = -ln(sigmoid(-y)); stage s = sigmoid(-y) now,
                # take ln + negate after all chunks (one table set at a time)
                nc.scalar.activation(alpha_b[b][0:1, cols], aps[:],
                                     mybir.ActivationFunctionType.Sigmoid,
                                     bias=projb2n[:], scale=-1.0)
                if ncol > 0:
                    nc.vector.tensor_scalar(
                        cs_b[b][0:1, j * 512:j * 512 + ncol], aps[:, 0:ncol],
                        thetmb[:], None, mybir.AluOpType.is_gt)

        # tail per batch: l = ln(s) (in place); qty sums from l directly via
        # sigmoid((-l-1)/0.1); alpha = -l (in place); DMA out; fires cumsum.
        for b in range(NB):
            nc.scalar.activation(alpha_b[b][:], alpha_b[b][:],
                                 mybir.ActivationFunctionType.Ln,
                                 bias=zero128[0:1, :], scale=1.0)
        for b in range(NB):
            osum_sb = work.tile([1, 1], F32, tag="osum_sb")
            # dump the sigmoid values into the (now dead) zT buffer; only the
            # accumulated sum is consumed.
            nc.scalar.activation(zTb[b][0:1, 0:T], alpha_b[b][:, 0:T],
                                 mybir.ActivationFunctionType.Sigmoid,
                                 bias=negten[0:1, :], scale=-10.0,
                                 accum_out=osum_sb[:])
            nc.sync.dma_start(out=osum[b:b + 1, :], in_=osum_sb[:])
            nc.vector.tensor_scalar_mul(alpha_b[b][:], alpha_b[b][:], -1.0)
            nc.sync.dma_start(out=alpha_o[b:b + 1, :], in_=alpha_b[b][:, 0:T])
            nc.vector.tensor_tensor_scan(cs_b[b][:], cs_b[b][:], cs_b[b][:], 0.0,
                                         mybir.AluOpType.add,
                                         mybir.AluOpType.bypass)

        # transpose cumsum -> [t-part, (tile, b)]
        cstp = ps_sm.tile([128, N_TTILES * NB], F32, tag="sm")
        for k in range(N_TTILES):
            ncol = 128 if k < N_TTILES - 1 else LAST_T
            for b in range(NB):
                nc.tensor.transpose(cstp[0:ncol, k * NB + b:k * NB + b + 1],
                                    cs_b[b][:, k * 128:k * 128 + ncol],
                                    ident_sb[0:1, 0:1])
        csT = abuf.tile([128, N_TTILES * NB], F32)
        # rows of the last (short) tile beyond LAST_T must not contribute:
        # pre-fill with large cumsum so slot > cs is false there.
        nc.vector.memset(csT[:], 1e9)
        nc.vector.tensor_copy(csT[:, 0:(N_TTILES - 1) * NB],
                              cstp[:, 0:(N_TTILES - 1) * NB])
        nc.vector.tensor_copy(csT[0:LAST_T, (N_TTILES - 1) * NB:N_TTILES * NB],
                              cstp[0:LAST_T, (N_TTILES - 1) * NB:N_TTILES * NB])

        # fire_w = min(floor(searchsorted/8), 374) via mask8 matmul counts.
        # mask8 is the stationary operand (one LDWEIGHTS per batch group);
        # the [1, NF] count row is transposed back to a column afterwards.
        idxs = []
        for b in range(NB):
            fwp = ps_sm.tile([1, NF], F32, tag="fwacc", bufs=1, name=f"fwp{b}")
            for k in range(N_TTILES):
                isl = work.tile([128, NF], BF16, tag="isl")
                nc.vector.tensor_tensor(
                    isl[:], slotv_sb[:], csT[:, k * NB + b:k * NB + b + 1]
                    .to_broadcast([128, NF]), mybir.AluOpType.is_gt)
                nc.tensor.matmul(fwp[:], lhsT=mask8_sb[:], rhs=isl[:],
                                 start=(k == 0), stop=(k == N_TTILES - 1))
            fwf = work.tile([1, NF], F32, tag="fwf")
            nc.vector.tensor_scalar_min(fwf[:], fwp[:], 374.0)
            fwtp = ps_sm.tile([NF, 1], F32, tag="sm", name=f"fwtp{b}")
            nc.tensor.transpose(fwtp[:], fwf[:], ident_sb[0:1, 0:1])
            fwi = work.tile([NF, 1], I32, tag="fwi")
            nc.vector.tensor_copy(fwi[:], fwtp[:])
            idx_sb = sgp.tile([NF, H], I32, tag="idx")
            nc.vector.tensor_tensor(idx_sb[:], hoff_sb[:, b * H:(b + 1) * H],
                                    fwi[:].to_broadcast([NF, H]),
                                    mybir.AluOpType.add)
            idxs.append(idx_sb)

        # ================= phase B: swin =================
        swin_flat = swin.ap().rearrange("b h w d -> (b h w) d")
        # HW indirect DMA consumes exactly one index per output partition, so
        # gather one h-slice ([128 fires, 192]) per call.
        for b in (() if skip_swin else range(NB)):
            for h in range(H):
                sg = sgp.tile([NF, DSW], F32, tag="sg", name=f"sg{b}_{h}", bufs=8)
                nc.gpsimd.indirect_dma_start(
                    out=sg[:], out_offset=None, in_=swin_flat,
                    in_offset=IndirectOffsetOnAxis(
                        ap=idxs[b][:, h:h + 1], axis=0))
                st1p = ps_tr.tile([128, 128], F32, tag="tr")
                nc.tensor.transpose(st1p[:], sg[:, 0:128], ident_sb[:])
                st2p = ps_tr.tile([64, 128], F32, tag="tr")
                nc.tensor.transpose(st2p[:], sg[:, 128:DSW], ident_sb[:])
                st1s = work.tile([128, 128], F32R, tag="st1s")
                nc.vector.tensor_copy(st1s[:], st1p[:])
                st65 = work.tile([65, 128], F32R, tag="st65")
                nc.vector.tensor_copy(st65[0:64, :], st2p[:])
                nc.vector.tensor_copy(st65[64:65, :], onesr_sb[:])
                swps = ps_big.tile([NF, DM], F32, tag="big")
                nc.tensor.matmul(swps[:], lhsT=_mm_dt(st1s[:], USE_F32R_BIG),
                                 rhs=_mm_dt(swA_sb[:], USE_F32R_BIG),
                                 start=True, stop=False)
                nc.tensor.matmul(swps[:], lhsT=_mm_dt(st65[:], USE_F32R_BIG),
                                 rhs=_mm_dt(swB65_sb[:], USE_F32R_BIG),
                                 start=False, stop=True)
                swo = outp.tile([NF, DM], F32, tag="out")
                nc.scalar.copy(swo[:], swps[:])
                nc.sync.dma_start(
                    out=bass.AP(tensor=acoustic,
                                offset=(b * NF * (PP + H) + PP + h) * DM,
                                ap=[[(PP + H) * DM, NF], [1, DM]]),
                    in_=swo[:])

    nc.finalize()
    return nc


_NC_CACHE = None


def _get_nc():
    global _NC_CACHE
    if _NC_CACHE is None:
        _NC_CACHE = build_nc()
    return _NC_CACHE


def make_constants():
    ident = np.eye(128, dtype=np.float32)
    slotv = np.broadcast_to(np.arange(1, NF + 1, dtype=np.float32)[None, :],
                            (128, NF)).copy()
    import ml_dtypes
    m8 = np.zeros((128, 1), dtype=ml_dtypes.bfloat16)
    m8[7::8, 0] = 1.0
    hoff = np.zeros((NB, NF, H), dtype=np.int32)
    for b in range(NB):
        hoff[b, :, :] = (W * (H * b + np.arange(H)))[None, :]
    return ident, slotv, m8, hoff


def kernel(fire_signal, swin_2d, pitch_tokens, target_lengths,
           conv_w, ln_w, ln_b, dense_w, dense_b, proj_w, proj_b,
           pitch_w, pitch_b, swin_w, swin_b):
    from concourse.bass_utils import run_bass_kernel_spmd

    target_lengths = np.asarray(target_lengths)
    inputs = dict(fire_signal=fire_signal, swin_2d=swin_2d,
                  pitch_tokens=pitch_tokens, conv_w=conv_w, ln_w=ln_w,
                  ln_b=ln_b, dense_w=dense_w, dense_b=dense_b, proj_w=proj_w,
                  proj_b=proj_b, pitch_w=pitch_w, pitch_b=pitch_b,
                  swin_w=swin_w, swin_b=swin_b)
    in_maps = _build_in_maps(inputs)
    n_cores = 8

    nc = _get_nc()
    res = run_bass_kernel_spmd(nc, in_maps, list(range(n_cores)))
    outs = res.results

    acoustic = np.concatenate([outs[c]["acoustic"] for c in range(n_cores)], axis=0)
    alpha = np.concatenate([outs[c]["alpha_o"] for c in range(n_cores)], axis=0)
    osums = np.concatenate([outs[c]["osum"][:, 0] for c in range(n_cores)], axis=0)
    qty = np.float32(np.mean(np.abs(osums - target_lengths.astype(np.float32))))
    return acoustic, alpha, qty


def _build_in_maps(inputs):
    """Shard + pack full inputs into per-core in_maps (same as kernel())."""
    fire_signal = np.asarray(inputs["fire_signal"], np.float32)
    conv_w = np.asarray(inputs["conv_w"], np.float32)
    ident, slotv, m8, hoff = make_constants()
    convw_t = np.ascontiguousarray(conv_w[:, 0, :].T)
    base = {
        "convw": convw_t,
        "lnw": np.asarray(inputs["ln_w"], np.float32).reshape(C, 1),
        "lnb": np.asarray(inputs["ln_b"], np.float32).reshape(C, 1),
        "dw": np.ascontiguousarray(np.asarray(inputs["dense_w"], np.float32)),
        "db": np.asarray(inputs["dense_b"], np.float32).reshape(HID, 1),
        "pw": np.ascontiguousarray(np.asarray(inputs["proj_w"], np.float32)).reshape(HID, 1),
        "projb2": np.full((NB, 1), np.float32(np.asarray(inputs["proj_b"]).reshape(-1)[0]), np.float32),
        "pw33": np.concatenate([np.asarray(inputs["pitch_w"], np.float32),
                                np.asarray(inputs["pitch_b"], np.float32).reshape(1, DM)], 0),
        "swA": np.ascontiguousarray(np.asarray(inputs["swin_w"], np.float32)[0:128]),
        "swB65": np.concatenate([np.asarray(inputs["swin_w"], np.float32)[128:DSW],
                                 np.asarray(inputs["swin_b"], np.float32).reshape(1, DM)], 0),
        "ident": ident, "slotv": slotv, "mask8": m8, "hoff": hoff,
        "zrow": np.zeros((1, C), np.float32),
        "onesr": np.ones((1, 128), np.float32),
    }
    maps = []
    for c in range(8):
        bs = slice(c * NB, (c + 1) * NB)
        m = dict(base)
        m["fire"] = np.ascontiguousarray(fire_signal[bs])
        m["swin"] = np.ascontiguousarray(np.asarray(inputs["swin_2d"], np.float32)[bs])
        m["pitch"] = np.ascontiguousarray(np.asarray(inputs["pitch_tokens"], np.float32)[bs])
        maps.append(m)
    return maps


def timed_run(inputs, iters=6, nc=None, verbose=False):
    """Steady-state per-launch wall time of the 8-core SPMD kernel, in ns.

    Jits once, keeps inputs on device, feeds each run's outputs back as the
    next run's donated output buffers so no host transfers land in the timed
    region.
    """
    import time
    import jax
    from jax.sharding import Mesh, PartitionSpec
    from jax.experimental.shard_map import shard_map
    from concourse import bass2jax
    from concourse.bass2jax import _bass_exec_p, partition_id_tensor
    import concourse.mybir as mybir_

    if nc is None:
        nc = _get_nc()
    bass2jax.install_neuronx_cc_hook()
    in_maps = _build_in_maps(inputs)
    n_cores = 8

    partition_name = nc.partition_id_tensor.name if nc.partition_id_tensor else None
    in_names, out_names, out_avals, zero_outs = [], [], [], []
    for alloc in nc.m.functions[0].allocations:
        if not isinstance(alloc, mybir_.MemoryLocationSet):
            continue
        name = alloc.memorylocations[0].name
        if alloc.kind == "ExternalInput":
            if name != partition_name:
                in_names.append(name)
        elif alloc.kind == "ExternalOutput":
            shape = tuple(alloc.tensor_shape)
            dtype = mybir_.dt.np(alloc.dtype)
            out_names.append(name)
            out_avals.append(jax.core.ShapedArray(shape, dtype))
            zero_outs.append(np.zeros(shape, dtype))
    n_params = len(in_names)
    n_outs = len(out_avals)
    all_in_names = in_names + out_names + ([partition_name] if partition_name else [])

    def _body(*args):
        operands = list(args)
        if partition_name is not None:
            operands.append(partition_id_tensor())
        outs = _bass_exec_p.bind(
            *operands, out_avals=tuple(out_avals), in_names=tuple(all_in_names),
            out_names=tuple(out_names), lowering_input_output_aliases=(),
            sim_require_finite=True, sim_require_nnan=True, nc=nc)
        return tuple(outs)

    devices = jax.devices()[:n_cores]
    mesh = Mesh(np.asarray(devices), ("core",))
    in_specs = (PartitionSpec("core"),) * (n_params + n_outs)
    out_specs = (PartitionSpec("core"),) * n_outs
    donate = tuple(range(n_params, n_params + n_outs))
    sharded = jax.jit(
        shard_map(_body, mesh=mesh, in_specs=in_specs, out_specs=out_specs,
                  check_rep=False),
        donate_argnums=donate, keep_unused=True)

    concat_in = [np.concatenate([np.asarray(in_maps[c][n]) for c in range(n_cores)], 0)
                 for n in in_names]
    cur_outs = [np.zeros((n_cores * z.shape[0], *z.shape[1:]), z.dtype)
                for z in zero_outs]
    sharding = jax.sharding.NamedSharding(mesh, PartitionSpec("core"))
    dev_in = [jax.device_put(a, sharding) for a in concat_in]
    cur_outs = [jax.device_put(a, sharding) for a in cur_outs]

    times = []
    for i in range(iters):
        t0 = time.perf_counter()
        res = sharded(*dev_in, *cur_outs)
        jax.block_until_ready(res)
        t1 = time.perf_counter()
        times.append(t1 - t0)
        cur_outs = list(res)
    if verbose:
        print("iter times (ms):", [f"{t*1e3:.2f}" for t in times])
    times = sorted(times[1:])  # drop compile/warmup iteration
    med = times[len(times) // 2]
    return int(med * 1e9)
